# revision 1
# baseline (speedup 1.0000x reference)
"""Multi-head attention block (b=4, n=2048, d=256, h=8) on 8 TRN2 NeuronCores.

Sharding: core c handles (batch bi=c//2, query-half qh=c%2): it computes
K/V for the full sequence of its batch and Q for its 1024-row query half,
producing 1024 complete rows of the final output (no cross-core reduction;
host just concatenates and adds b_out).

Per-core device program (all matmuls fp32r, single-tile position only):
  1. QKV^T projections from host-pretransposed x^T:
     qT[h] [32,1024], kT[h] [32,2048] per head; v_aug [128,16,8*33]
     ([v|ones] per head per k-tile; the ones column folds the softmax
     denominator into the AV matmul as output row 32).
  2. Attention per (head-group hg, q-chunk qc of 512):
     scores^T k-tile [128,512] on PE -> exp on ACT (pairs of heads share a
     2-bank psum so exp runs at N=1024) -> fp32r probs in SBUF ->
     AV accumulate [33,512] psum over the 16 k-tiles.
  3. Normalize: denominator rows -> DRAM bounce -> batched DVE reciprocal
     [128,32] -> broadcast-read [32,512] -> DVE multiply -> outT.
  4. Output projection outT^T @ w_out -> out rows -> DRAM.
"""
import numpy as np

import concourse.bacc as bacc
import concourse.bass as bass
import concourse.mybir as mybir
import concourse.tile as tile
from concourse.bass_utils import run_bass_kernel_spmd

F32 = mybir.dt.float32
F32R = mybir.dt.float32r
Exp = mybir.ActivationFunctionType.Exp
Copy = mybir.ActivationFunctionType.Copy

B, N, D = 4, 2048, 256
H, DH = 8, 32
NQ = N // 2            # per-core query rows
SCALE = D ** -0.5      # 0.0625
NKT = N // 128         # 16 k-tiles
NQC = NQ // 512        # 2 q-chunks per core

_BUILD_CACHE = {}


def build():
    if "nc" in _BUILD_CACHE:
        return _BUILD_CACHE["nc"]
    nc = bacc.Bacc()

    xT_d = nc.dram_tensor("xT", [D, N], F32, kind="ExternalInput")
    xqT_d = nc.dram_tensor("xqT", [D, NQ], F32, kind="ExternalInput")
    w_d = nc.dram_tensor("w_qkv", [D, 3 * D], F32, kind="ExternalInput")
    b_d = nc.dram_tensor("b_qkv", [1, 3 * D], F32, kind="ExternalInput")
    wo_d = nc.dram_tensor("w_out", [D, D], F32, kind="ExternalInput")
    out_d = nc.dram_tensor("out", [NQ, D], F32, kind="ExternalOutput")
    den_dram = nc.dram_tensor("den_scratch", [H, NQC, 512], F32)
    recip_dram = nc.dram_tensor("recip_scratch", [H, NQC, 512], F32)

    with tile.TileContext(nc) as tc:
        with (
            tc.tile_pool(name="persist", bufs=1) as persist,
            tc.tile_pool(name="probs", bufs=4) as prpool,
            tc.tile_pool(name="avsb", bufs=8) as avsb_pool,
            tc.tile_pool(name="norm", bufs=4) as norm_pool,
            tc.tile_pool(name="outsb", bufs=3) as out_pool,
        ):
            # ---- constants / persistent tiles ----
            ones_st = persist.tile([1, 512], F32, name="ones_st")
            nc.vector.memset(ones_st, 1.0)
            ones = persist.tile([1, 512], F32R, name="ones")
            nc.vector.tensor_copy(ones, ones_st)

            kT = [persist.tile([32, N], F32R, name=f"kT{h}") for h in range(H)]
            qT = [persist.tile([32, NQ], F32R, name=f"qT{h}") for h in range(H)]
            v_aug = persist.tile([128, NKT, H * 33], F32R, name="v_aug")
            outT = [persist.tile([128, NQ], F32R, name=f"outT{g}") for g in range(2)]
            wo_sb = [persist.tile([128, D], F32R, name=f"wo{g}") for g in range(2)]
            for g in range(2):
                nc.gpsimd.dma_start(out=wo_sb[g], in_=wo_d[128 * g:128 * (g + 1), :])

            # ---- phase A: QKV projections ----
            with (
                tc.tile_pool(name="stage_a", bufs=1) as pa,
                tc.tile_pool(name="vstage", bufs=3) as vstage_pool,
                tc.tile_pool(name="qkvps", bufs=3, space="PSUM") as kqps,
                tc.tile_pool(name="vps", bufs=2, space="PSUM") as vps,
            ):
                w_sb = [pa.tile([128, 3 * D], F32R, name=f"w{d2}") for d2 in range(2)]
                b_sb = pa.tile([1, 3 * D], F32R, name="b_sb")
                xT_sb = [pa.tile([128, N], F32R, name=f"xT{d2}") for d2 in range(2)]
                xqT_sb = [pa.tile([128, NQ], F32R, name=f"xq{d2}") for d2 in range(2)]
                for d2 in range(2):
                    nc.gpsimd.dma_start(out=w_sb[d2], in_=w_d[128 * d2:128 * (d2 + 1), :])
                    nc.gpsimd.dma_start(out=xqT_sb[d2], in_=xqT_d[128 * d2:128 * (d2 + 1), :])
                    nc.gpsimd.dma_start(out=xT_sb[d2], in_=xT_d[128 * d2:128 * (d2 + 1), :])
                nc.gpsimd.dma_start(out=b_sb, in_=b_d[:, :])

                def proj_cols(dst_tiles, col0, rhs_tiles, seq0, nseq, tag):
                    """qkv^T chunk: out[128 dims, nseq] = w[:,cols].T @ x^T chunk
                    + bias, then per-head copies into dst[h][:, seq0:...]."""
                    p = kqps.tile([128, 512], F32, tag="kq", name=f"kq_{tag}")
                    for d2 in range(2):
                        nc.tensor.matmul(
                            p[:, :nseq],
                            w_sb[d2][:, col0:col0 + 128],
                            rhs_tiles[d2][:, seq0:seq0 + nseq],
                            start=(d2 == 0), stop=False,
                        )
                    nc.tensor.matmul(
                        p[:, :nseq], b_sb[:, col0:col0 + 128], ones[:, :nseq],
                        start=False, stop=True,
                    )
                    for j in range(4):
                        # alternate DVE / ACT for psum evacuation
                        eng = nc.vector.tensor_copy if j % 2 == 0 else (
                            lambda out, in_: nc.scalar.activation(
                                out=out, in_=in_, func=Copy))
                        eng(out=dst_tiles[j][:, seq0:seq0 + nseq],
                            in_=p[32 * j:32 * (j + 1), :nseq])

                for hg in range(2):
                    for c in range(NQ // 512):
                        proj_cols(qT[4 * hg:4 * hg + 4], 128 * hg, xqT_sb,
                                  512 * c, 512, f"q{hg}_{c}")
                    for c in range(N // 512):
                        proj_cols(kT[4 * hg:4 * hg + 4], D + 128 * hg, xT_sb,
                                  512 * c, 512, f"k{hg}_{c}")

                for st in range(NKT):
                    pv = vps.tile([128, D], F32, tag="v", name=f"v_{st}")
                    for d2 in range(2):
                        nc.tensor.matmul(
                            pv[:, :],
                            xT_sb[d2][:, 128 * st:128 * (st + 1)],
                            w_sb[d2][:, 2 * D:3 * D],
                            start=(d2 == 0), stop=False,
                        )
                    nc.tensor.matmul(
                        pv[:, :], ones[:, :128], b_sb[:, 2 * D:3 * D],
                        start=False, stop=True,
                    )
                    stg = vstage_pool.tile([128, H * 33], F32, tag="vst",
                                           name=f"vst_{st}")
                    nc.vector.memset(stg, 1.0)
                    nc.vector.tensor_copy(
                        out=stg.rearrange("p (h c) -> p h c", h=H)[:, :, 0:32],
                        in_=pv.rearrange("p (h c) -> p h c", h=H),
                    )
                    nc.vector.tensor_copy(out=v_aug[:, st, :], in_=stg)

            # ---- phase B: attention ----
            with (
                tc.tile_pool(name="scps", bufs=2, space="PSUM") as scps,
                tc.tile_pool(name="avps", bufs=4, space="PSUM") as avps,
            ):
                for hg in range(2):
                    av_sb_all = {}
                    for qc in range(NQC):
                        av = [avps.tile([33, 512], F32, tag="av",
                                        name=f"av_{hg}_{qc}_{j}") for j in range(4)]

                        def emit_av(prs, kt):
                            for p in range(2):
                                for i in range(2):
                                    j = 2 * p + i
                                    h = 4 * hg + j
                                    nc.tensor.matmul(
                                        av[j][:, :],
                                        v_aug[:, kt, 33 * h:33 * h + 33],
                                        prs[p][:, 512 * i:512 * (i + 1)],
                                        start=(kt == 0), stop=(kt == NKT - 1),
                                    )

                        prev = None
                        for kt in range(NKT):
                            cur = []
                            for p in range(2):
                                S = scps.tile([128, 2, 512], F32, tag="S",
                                              name=f"S_{hg}_{qc}_{kt}_{p}")
                                for i in range(2):
                                    h = 4 * hg + 2 * p + i
                                    nc.tensor.matmul(
                                        S[:, i, :],
                                        kT[h][:, 128 * kt:128 * (kt + 1)],
                                        qT[h][:, 512 * qc:512 * (qc + 1)],
                                        start=True, stop=True,
                                    )
                                pr = prpool.tile([128, 1024], F32R, tag="pr",
                                                 name=f"pr_{hg}_{qc}_{kt}_{p}")
                                nc.scalar.activation(out=pr, in_=S[:, :, :],
                                                     func=Exp, scale=SCALE)
                                cur.append(pr)
                            if prev is not None:
                                emit_av(prev, kt - 1)
                            prev = cur
                        emit_av(prev, NKT - 1)

                        for j in range(4):
                            h = 4 * hg + j
                            a = avsb_pool.tile([33, 512], F32, tag="avsb",
                                               name=f"avsb_{hg}_{qc}_{j}")
                            nc.vector.tensor_copy(a, av[j][:, :])
                            nc.sync.dma_start(out=den_dram[h, qc, :],
                                              in_=a[32:33, :])
                            av_sb_all[(qc, j)] = a

                    # batched reciprocal for this head-group (4h x NQC x 512)
                    nchunk = 4 * NQC * 512 // 128  # 32
                    denb = norm_pool.tile([128, nchunk], F32, tag="denb",
                                          name=f"denb{hg}")
                    nc.sync.dma_start(
                        out=denb,
                        in_=den_dram[4 * hg:4 * hg + 4, :, :]
                        .rearrange("a b c -> (a b c)")
                        .rearrange("(p f) -> p f", p=128),
                    )
                    recb = norm_pool.tile([128, nchunk], F32, tag="recb",
                                          name=f"recb{hg}")
                    nc.vector.reciprocal(recb, denb)
                    nc.sync.dma_start(
                        out=recip_dram[4 * hg:4 * hg + 4, :, :]
                        .rearrange("a b c -> (a b c)")
                        .rearrange("(p f) -> p f", p=128),
                        in_=recb,
                    )
                    for qc in range(NQC):
                        for j in range(4):
                            h = 4 * hg + j
                            row = recip_dram[h, qc, :]
                            bc = norm_pool.tile([32, 512], F32, tag="bc",
                                                name=f"bc_{hg}_{qc}_{j}")
                            nc.gpsimd.dma_start(
                                out=bc,
                                in_=bass.AP(tensor=row.tensor, offset=row.offset,
                                            ap=[[0, 32], row.ap[-1]]),
                            )
                            nc.vector.tensor_mul(
                                outT[hg][32 * j:32 * (j + 1),
                                         512 * qc:512 * (qc + 1)],
                                av_sb_all[(qc, j)][0:32, :], bc)

            # ---- phase C: output projection ----
            with tc.tile_pool(name="ops", bufs=2, space="PSUM") as ops:
                for qt in range(NQ // 128):
                    po = ops.tile([128, D], F32, tag="po", name=f"po{qt}")
                    for g in range(2):
                        nc.tensor.matmul(
                            po[:, :],
                            outT[g][:, 128 * qt:128 * (qt + 1)],
                            wo_sb[g][:, :],
                            start=(g == 0), stop=(g == 1),
                        )
                    o = out_pool.tile([128, D], F32, tag="o", name=f"o{qt}")
                    nc.vector.tensor_copy(o, po[:, :])
                    nc.sync.dma_start(out=out_d[128 * qt:128 * (qt + 1), :], in_=o)

    nc.compile()
    _BUILD_CACHE["nc"] = nc
    return nc


def _run(x, w_qkv, b_qkv, w_out, trace=False):
    nc = build()
    in_maps = []
    for c in range(8):
        bi, qh = c // 2, c % 2
        in_maps.append({
            "xT": np.ascontiguousarray(x[bi].T),
            "xqT": np.ascontiguousarray(x[bi, NQ * qh:NQ * (qh + 1)].T),
            "w_qkv": np.ascontiguousarray(w_qkv),
            "b_qkv": np.ascontiguousarray(b_qkv.reshape(1, 3 * D)),
            "w_out": np.ascontiguousarray(w_out),
        })
    res = run_bass_kernel_spmd(nc, in_maps, core_ids=list(range(8)), trace=trace)
    out = np.empty((B, N, D), dtype=np.float32)
    for c in range(8):
        bi, qh = c // 2, c % 2
        out[bi, NQ * qh:NQ * (qh + 1)] = res.results[c]["out"]
    return out, res


def kernel(x, w_qkv, b_qkv, w_out, b_out):
    x = np.asarray(x, dtype=np.float32)
    out, _ = _run(x, np.asarray(w_qkv, np.float32), np.asarray(b_qkv, np.float32),
                  np.asarray(w_out, np.float32))
    return out + np.asarray(b_out, np.float32)[None, None, :]


# revision 2
# speedup vs baseline: 1.0961x; 1.0961x over previous
"""Multi-head attention block (b=4, n=2048, d=256, h=8) on 8 TRN2 NeuronCores.

Sharding: core c handles (batch bi=c//2, query-half qh=c%2): it computes
K/V for the full sequence of its batch and Q for its 1024-row query half,
producing 1024 complete rows of the final output (no cross-core reduction;
host just concatenates and adds b_out).

Per-core device program (all matmuls fp32r, single-tile position only):
  1. QKV^T projections from host-pretransposed x^T:
     qT[h] [32,1024], kT[h] [32,2048] per head; v_aug [128,16,8*33]
     ([v|ones] per head per k-tile; the ones column folds the softmax
     denominator into the AV matmul as output row 32).
  2. Attention per (head-group hg, q-chunk qc of 512):
     scores^T k-tile [128,512] on PE -> exp on ACT (pairs of heads share a
     2-bank psum so exp runs at N=1024) -> fp32r probs in SBUF ->
     AV accumulate [33,512] psum over the 16 k-tiles.
  3. Normalize: denominator rows -> DRAM bounce -> batched DVE reciprocal
     [128,32] -> broadcast-read [32,512] -> DVE multiply -> outT.
  4. Output projection outT^T @ w_out -> out rows -> DRAM.
"""
import numpy as np

import concourse.bacc as bacc
import concourse.bass as bass
import concourse.mybir as mybir
import concourse.tile as tile
from concourse.bass_utils import run_bass_kernel_spmd

F32 = mybir.dt.float32
F32R = mybir.dt.float32r
F16 = mybir.dt.float16
Exp = mybir.ActivationFunctionType.Exp
Copy = mybir.ActivationFunctionType.Copy

B, N, D = 4, 2048, 256
H, DH = 8, 32
NQ = N // 2            # per-core query rows
SCALE = D ** -0.5      # 0.0625
NKT = N // 128         # 16 k-tiles
NQC = NQ // 512        # 2 q-chunks per core

_BUILD_CACHE = {}


def build():
    if "nc" in _BUILD_CACHE:
        return _BUILD_CACHE["nc"]
    nc = bacc.Bacc()

    xT_d = nc.dram_tensor("xT", [D, N], F32, kind="ExternalInput")
    xqT_d = nc.dram_tensor("xqT", [D, NQ], F32, kind="ExternalInput")
    w_d = nc.dram_tensor("w_qkv", [D, 3 * D], F32, kind="ExternalInput")
    b_d = nc.dram_tensor("b_qkv", [1, 3 * D], F32, kind="ExternalInput")
    wo_d = nc.dram_tensor("w_out", [D, D], F32, kind="ExternalInput")
    out_d = nc.dram_tensor("out", [NQ, D], F32, kind="ExternalOutput")
    den_dram = nc.dram_tensor("den_scratch", [H, NQC, 512], F32)
    recip_dram = nc.dram_tensor("recip_scratch", [H, NQC, 512], F32)

    with tile.TileContext(nc) as tc:
        with (
            tc.tile_pool(name="persist", bufs=1) as persist,
            tc.tile_pool(name="probs", bufs=4) as prpool,
            tc.tile_pool(name="avsb", bufs=8) as avsb_pool,
            tc.tile_pool(name="norm", bufs=4) as norm_pool,
            tc.tile_pool(name="outsb", bufs=3) as out_pool,
        ):
            # ---- constants / persistent tiles ----
            ones_st = persist.tile([1, 512], F32, name="ones_st")
            nc.vector.memset(ones_st, 1.0)
            ones = persist.tile([1, 512], F16, name="ones")
            nc.vector.tensor_copy(ones, ones_st)

            kT = [persist.tile([32, N], F16, name=f"kT{h}") for h in range(H)]
            qT = [persist.tile([32, NQ], F16, name=f"qT{h}") for h in range(H)]
            v_aug = persist.tile([128, NKT, H * 33], F16, name="v_aug")
            outT = [persist.tile([128, NQ], F16, name=f"outT{g}") for g in range(2)]
            wo_sb = [persist.tile([128, D], F16, name=f"wo{g}") for g in range(2)]
            for g in range(2):
                nc.gpsimd.dma_start(out=wo_sb[g], in_=wo_d[128 * g:128 * (g + 1), :])

            # ---- phase A: QKV projections ----
            with (
                tc.tile_pool(name="stage_a", bufs=1) as pa,
                tc.tile_pool(name="vstage", bufs=3) as vstage_pool,
                tc.tile_pool(name="qkvps", bufs=3, space="PSUM") as kqps,
                tc.tile_pool(name="vps", bufs=2, space="PSUM") as vps,
            ):
                w_sb = [pa.tile([128, 3 * D], F16, name=f"w{d2}") for d2 in range(2)]
                b_sb = pa.tile([1, 3 * D], F16, name="b_sb")
                xT_sb = [pa.tile([128, N], F16, name=f"xT{d2}") for d2 in range(2)]
                xqT_sb = [pa.tile([128, NQ], F16, name=f"xq{d2}") for d2 in range(2)]
                for d2 in range(2):
                    nc.gpsimd.dma_start(out=w_sb[d2], in_=w_d[128 * d2:128 * (d2 + 1), :])
                    nc.gpsimd.dma_start(out=xqT_sb[d2], in_=xqT_d[128 * d2:128 * (d2 + 1), :])
                    nc.gpsimd.dma_start(out=xT_sb[d2], in_=xT_d[128 * d2:128 * (d2 + 1), :])
                nc.gpsimd.dma_start(out=b_sb, in_=b_d[:, :])

                def proj_cols(dst_tiles, col0, rhs_tiles, seq0, nseq, tag):
                    """qkv^T chunk: out[128 dims, nseq] = w[:,cols].T @ x^T chunk
                    + bias, then per-head copies into dst[h][:, seq0:...]."""
                    p = kqps.tile([128, 512], F32, tag="kq", name=f"kq_{tag}")
                    for d2 in range(2):
                        nc.tensor.matmul(
                            p[:, :nseq],
                            w_sb[d2][:, col0:col0 + 128],
                            rhs_tiles[d2][:, seq0:seq0 + nseq],
                            start=(d2 == 0), stop=False,
                        )
                    nc.tensor.matmul(
                        p[:, :nseq], b_sb[:, col0:col0 + 128], ones[:, :nseq],
                        start=False, stop=True,
                    )
                    for j in range(4):
                        # alternate DVE / ACT for psum evacuation
                        eng = nc.vector.tensor_copy if j % 2 == 0 else (
                            lambda out, in_: nc.scalar.activation(
                                out=out, in_=in_, func=Copy))
                        eng(out=dst_tiles[j][:, seq0:seq0 + nseq],
                            in_=p[32 * j:32 * (j + 1), :nseq])

                for hg in range(2):
                    for c in range(NQ // 512):
                        proj_cols(qT[4 * hg:4 * hg + 4], 128 * hg, xqT_sb,
                                  512 * c, 512, f"q{hg}_{c}")
                    for c in range(N // 512):
                        proj_cols(kT[4 * hg:4 * hg + 4], D + 128 * hg, xT_sb,
                                  512 * c, 512, f"k{hg}_{c}")

                for st in range(NKT):
                    pv = vps.tile([128, D], F32, tag="v", name=f"v_{st}")
                    for d2 in range(2):
                        nc.tensor.matmul(
                            pv[:, :],
                            xT_sb[d2][:, 128 * st:128 * (st + 1)],
                            w_sb[d2][:, 2 * D:3 * D],
                            start=(d2 == 0), stop=False,
                        )
                    nc.tensor.matmul(
                        pv[:, :], ones[:, :128], b_sb[:, 2 * D:3 * D],
                        start=False, stop=True,
                    )
                    stg = vstage_pool.tile([128, H * 33], F32, tag="vst",
                                           name=f"vst_{st}")
                    nc.vector.memset(stg, 1.0)
                    nc.vector.tensor_copy(
                        out=stg.rearrange("p (h c) -> p h c", h=H)[:, :, 0:32],
                        in_=pv.rearrange("p (h c) -> p h c", h=H),
                    )
                    nc.vector.tensor_copy(out=v_aug[:, st, :], in_=stg)

            # ---- phase B: attention ----
            with (
                tc.tile_pool(name="scps", bufs=2, space="PSUM") as scps,
                tc.tile_pool(name="avps", bufs=4, space="PSUM") as avps,
            ):
                for hg in range(2):
                    av_sb_all = {}
                    for qc in range(NQC):
                        av = [avps.tile([33, 512], F32, tag="av",
                                        name=f"av_{hg}_{qc}_{j}") for j in range(4)]

                        def emit_av(prs, kt):
                            for p in range(2):
                                for i in range(2):
                                    j = 2 * p + i
                                    h = 4 * hg + j
                                    nc.tensor.matmul(
                                        av[j][:, :],
                                        v_aug[:, kt, 33 * h:33 * h + 33],
                                        prs[p][:, 512 * i:512 * (i + 1)],
                                        start=(kt == 0), stop=(kt == NKT - 1),
                                    )

                        prev = None
                        for kt in range(NKT):
                            cur = []
                            for p in range(2):
                                S = scps.tile([128, 2, 512], F32, tag="S",
                                              name=f"S_{hg}_{qc}_{kt}_{p}")
                                for i in range(2):
                                    h = 4 * hg + 2 * p + i
                                    nc.tensor.matmul(
                                        S[:, i, :],
                                        kT[h][:, 128 * kt:128 * (kt + 1)],
                                        qT[h][:, 512 * qc:512 * (qc + 1)],
                                        start=True, stop=True,
                                    )
                                pr = prpool.tile([128, 1024], F16, tag="pr",
                                                 name=f"pr_{hg}_{qc}_{kt}_{p}")
                                nc.scalar.activation(out=pr, in_=S[:, :, :],
                                                     func=Exp, scale=SCALE)
                                cur.append(pr)
                            if prev is not None:
                                emit_av(prev, kt - 1)
                            prev = cur
                        emit_av(prev, NKT - 1)

                        for j in range(4):
                            h = 4 * hg + j
                            a = avsb_pool.tile([33, 512], F32, tag="avsb",
                                               name=f"avsb_{hg}_{qc}_{j}")
                            nc.vector.tensor_copy(a, av[j][:, :])
                            nc.sync.dma_start(out=den_dram[h, qc, :],
                                              in_=a[32:33, :])
                            av_sb_all[(qc, j)] = a

                    # batched reciprocal for this head-group (4h x NQC x 512)
                    nchunk = 4 * NQC * 512 // 128  # 32
                    denb = norm_pool.tile([128, nchunk], F32, tag="denb",
                                          name=f"denb{hg}")
                    nc.sync.dma_start(
                        out=denb,
                        in_=den_dram[4 * hg:4 * hg + 4, :, :]
                        .rearrange("a b c -> (a b c)")
                        .rearrange("(p f) -> p f", p=128),
                    )
                    recb = norm_pool.tile([128, nchunk], F32, tag="recb",
                                          name=f"recb{hg}")
                    nc.vector.reciprocal(recb, denb)
                    nc.sync.dma_start(
                        out=recip_dram[4 * hg:4 * hg + 4, :, :]
                        .rearrange("a b c -> (a b c)")
                        .rearrange("(p f) -> p f", p=128),
                        in_=recb,
                    )
                    for qc in range(NQC):
                        for j in range(4):
                            h = 4 * hg + j
                            row = recip_dram[h, qc, :]
                            bc = norm_pool.tile([32, 512], F32, tag="bc",
                                                name=f"bc_{hg}_{qc}_{j}")
                            nc.gpsimd.dma_start(
                                out=bc,
                                in_=bass.AP(tensor=row.tensor, offset=row.offset,
                                            ap=[[0, 32], row.ap[-1]]),
                            )
                            nc.vector.tensor_mul(
                                outT[hg][32 * j:32 * (j + 1),
                                         512 * qc:512 * (qc + 1)],
                                av_sb_all[(qc, j)][0:32, :], bc)

            # ---- phase C: output projection ----
            with tc.tile_pool(name="ops", bufs=2, space="PSUM") as ops:
                for qt in range(NQ // 128):
                    po = ops.tile([128, D], F32, tag="po", name=f"po{qt}")
                    for g in range(2):
                        nc.tensor.matmul(
                            po[:, :],
                            outT[g][:, 128 * qt:128 * (qt + 1)],
                            wo_sb[g][:, :],
                            start=(g == 0), stop=(g == 1),
                        )
                    o = out_pool.tile([128, D], F32, tag="o", name=f"o{qt}")
                    nc.vector.tensor_copy(o, po[:, :])
                    nc.sync.dma_start(out=out_d[128 * qt:128 * (qt + 1), :], in_=o)

    nc.compile()
    _BUILD_CACHE["nc"] = nc
    return nc


def _run(x, w_qkv, b_qkv, w_out, trace=False):
    nc = build()
    in_maps = []
    for c in range(8):
        bi, qh = c // 2, c % 2
        in_maps.append({
            "xT": np.ascontiguousarray(x[bi].T),
            "xqT": np.ascontiguousarray(x[bi, NQ * qh:NQ * (qh + 1)].T),
            "w_qkv": np.ascontiguousarray(w_qkv),
            "b_qkv": np.ascontiguousarray(b_qkv.reshape(1, 3 * D)),
            "w_out": np.ascontiguousarray(w_out),
        })
    res = run_bass_kernel_spmd(nc, in_maps, core_ids=list(range(8)), trace=trace)
    out = np.empty((B, N, D), dtype=np.float32)
    for c in range(8):
        bi, qh = c // 2, c % 2
        out[bi, NQ * qh:NQ * (qh + 1)] = res.results[c]["out"]
    return out, res


def kernel(x, w_qkv, b_qkv, w_out, b_out):
    x = np.asarray(x, dtype=np.float32)
    out, _ = _run(x, np.asarray(w_qkv, np.float32), np.asarray(b_qkv, np.float32),
                  np.asarray(w_out, np.float32))
    return out + np.asarray(b_out, np.float32)[None, None, :]


# revision 7
# speedup vs baseline: 1.5891x; 1.4498x over previous
"""Multi-head attention block (b=4, n=2048, d=256, h=8) on 8 TRN2 NeuronCores.

Sharding: core c handles (batch bi=c//2, query-half qh=c%2): it computes
K/V for the full sequence of its batch and Q for its 1024-row query half,
producing 1024 complete rows of the final output (host concatenates and
adds b_out; no cross-core reduction).

V2 design (all matmul operands fp16; PSUM fp32):
  - kT_all[hg] [128,2048]: 4 heads' K^T stacked (compact, whole-psum copies).
  - qT_pad[h] [128,1024]: per-head Q^T zero-padded to 128 partitions, so the
    scores matmul runs K=128 (lhsT = kT_all chunk; the zero rows of qT_pad
    mask the other heads). Single tile-position, full-array matmuls only.
  - q-chunks of 256: one scores psum tensor [128,4,256] (2 banks) holds all
    4 heads of a head-group for one k-tile; ONE exp [128,1024] per k-tile.
  - AV: [v|ones] lhsT [128,33] folds the softmax denominator (row 32);
    av accumulators [33,2,256] share a bank per head-pair -> psum fits in
    4 (scores) + 2 (av) + 2 (projection) = 8 banks.
  - QKV/V projection units are woven into the attention emission stream as
    PE filler to keep the tensor engine dense (HAM clock at 2+ GHz).
  - Normalization: denominator rows -> DRAM bounce -> batched reciprocal
    [128,32] -> broadcast-read -> DVE multiply -> outT (fp16).
"""
import numpy as np

import concourse.bacc as bacc
import concourse.bass as bass
import concourse.mybir as mybir
import concourse.tile as tile
from concourse.bass_utils import run_bass_kernel_spmd

F32 = mybir.dt.float32
F16 = mybir.dt.float16
Exp = mybir.ActivationFunctionType.Exp
Copy = mybir.ActivationFunctionType.Copy

B, N, D = 4, 2048, 256
H, DH = 8, 32
NQ = N // 2            # per-core query rows
SCALE = D ** -0.5      # 0.0625
NKT = N // 128         # 16 k-tiles
QC = 256               # q-chunk
NQC = NQ // QC         # 4 q-chunks per core

_BUILD_CACHE = {}


def build():
    if "nc" in _BUILD_CACHE:
        return _BUILD_CACHE["nc"]
    nc = bacc.Bacc()

    xT_d = nc.dram_tensor("xT", [D, N], F32, kind="ExternalInput")
    xqT_d = nc.dram_tensor("xqT", [D, NQ], F32, kind="ExternalInput")
    w_d = nc.dram_tensor("w_qkv", [D, 3 * D], F32, kind="ExternalInput")
    b_d = nc.dram_tensor("b_qkv", [1, 3 * D], F32, kind="ExternalInput")
    wo_d = nc.dram_tensor("w_out", [D, D], F32, kind="ExternalInput")
    out_d = nc.dram_tensor("out", [NQ, D], F32, kind="ExternalOutput")
    den_dram = nc.dram_tensor("den_scratch", [2, NQC, 2, 512], F32)
    recip_dram = nc.dram_tensor("recip_scratch", [2, NQC, 2, 512], F32)

    with tile.TileContext(nc) as tc:
        with (
            tc.tile_pool(name="persist", bufs=1) as persist,
            tc.tile_pool(name="probs", bufs=4) as prpool,
            tc.tile_pool(name="avsb", bufs=8) as avsb_pool,
            tc.tile_pool(name="norm", bufs=4) as norm_pool,
            tc.tile_pool(name="outsb", bufs=3) as out_pool,
            tc.tile_pool(name="kqps", bufs=2, space="PSUM") as kqps,
            tc.tile_pool(name="scps", bufs=2, space="PSUM") as scps,
            tc.tile_pool(name="avps", bufs=1, space="PSUM") as avps,
        ):
            # ---- persistent tiles / loads ----
            ones = persist.tile([1, 512], F16, name="ones")
            nc.vector.memset(ones, 1.0)

            w_sb = [persist.tile([128, 3 * D], F16, name=f"w{d2}") for d2 in range(2)]
            b_sb = persist.tile([1, 3 * D], F16, name="b_sb")
            xT_sb = [persist.tile([128, N], F16, name=f"xT{d2}") for d2 in range(2)]
            xqT_sb = [persist.tile([128, NQ], F16, name=f"xq{d2}") for d2 in range(2)]
            wo_sb = [persist.tile([128, D], F16, name=f"wo{g}") for g in range(2)]
            for d2 in range(2):
                nc.gpsimd.dma_start(out=w_sb[d2], in_=w_d[128 * d2:128 * (d2 + 1), :])
                nc.gpsimd.dma_start(out=xqT_sb[d2], in_=xqT_d[128 * d2:128 * (d2 + 1), :])
            nc.gpsimd.dma_start(out=b_sb, in_=b_d[:, :])
            for d2 in range(2):
                nc.gpsimd.dma_start(out=xT_sb[d2], in_=xT_d[128 * d2:128 * (d2 + 1), :])
            for g in range(2):
                nc.gpsimd.dma_start(out=wo_sb[g], in_=wo_d[128 * g:128 * (g + 1), :])

            # per-chunk tiles: a chunk is fully written before first read, so
            # tile-granular RAW tracking cannot create emission-order cycles
            kT_c = [[persist.tile([128, 512], F16, name=f"kT{g}_{c}")
                     for c in range(4)] for g in range(2)]
            qT_pad = [persist.tile([128, NQ], F16, name=f"qT{h}") for h in range(H)]
            v_st = [persist.tile([128, H * 33], F16, name=f"vst{s}")
                    for s in range(NKT)]
            outT_c = [[persist.tile([128, 256], F16, name=f"outT{g}_{c}")
                       for c in range(NQC)] for g in range(2)]
            for h in range(H):
                nc.vector.memset(qT_pad[h], 0.0)
            for s in range(NKT):
                nc.vector.memset(v_st[s], 1.0)

            # ---- projection units (emitted woven into attention) ----
            def qT_unit(hg, c):
                """q^T for head-group hg, seq chunk c (512 wide)."""
                p = kqps.tile([128, 512], F32, tag="kq", name=f"kqq_{hg}_{c}")
                for d2 in range(2):
                    nc.tensor.matmul(
                        p[:, :], w_sb[d2][:, 128 * hg:128 * (hg + 1)],
                        xqT_sb[d2][:, 512 * c:512 * (c + 1)],
                        start=(d2 == 0), stop=False)
                nc.tensor.matmul(
                    p[:, :], b_sb[:, 128 * hg:128 * (hg + 1)], ones[:, :],
                    start=False, stop=True)
                for j in range(4):
                    nc.vector.tensor_copy(
                        out=qT_pad[4 * hg + j][32 * j:32 * (j + 1),
                                               512 * c:512 * (c + 1)],
                        in_=p[32 * j:32 * (j + 1), :])

            def kT_unit(hg, c):
                """k^T for head-group hg, seq chunk c (512 wide)."""
                p = kqps.tile([128, 512], F32, tag="kq", name=f"kqk_{hg}_{c}")
                for d2 in range(2):
                    nc.tensor.matmul(
                        p[:, :], w_sb[d2][:, D + 128 * hg:D + 128 * (hg + 1)],
                        xT_sb[d2][:, 512 * c:512 * (c + 1)],
                        start=(d2 == 0), stop=False)
                nc.tensor.matmul(
                    p[:, :], b_sb[:, D + 128 * hg:D + 128 * (hg + 1)], ones[:, :],
                    start=False, stop=True)
                nc.scalar.activation(out=kT_c[hg][c][:, :], in_=p[:, :], func=Copy)

            def v_unit(st):
                """v rows for seq tile st (128 wide), all 8 heads + ones col."""
                p = kqps.tile([128, D], F32, tag="kq", name=f"vv_{st}")
                for d2 in range(2):
                    nc.tensor.matmul(
                        p[:, :], xT_sb[d2][:, 128 * st:128 * (st + 1)],
                        w_sb[d2][:, 2 * D:3 * D],
                        start=(d2 == 0), stop=False)
                nc.tensor.matmul(
                    p[:, :], ones[:, :128], b_sb[:, 2 * D:3 * D],
                    start=False, stop=True)
                nc.vector.tensor_copy(
                    out=v_st[st].rearrange("p (h c) -> p h c", h=H)[:, :, 0:32],
                    in_=p.rearrange("p (h c) -> p h c", h=H))

            # ---- attention ----
            for hg in range(2):
                av_sb_all = {}
                for qc in range(NQC):
                    av2 = [avps.tile([33, 2, 256], F32, tag=f"av{p}",
                                     name=f"av_{hg}_{qc}_{p}") for p in range(2)]

                    def emit_av(pr, kt):
                        for j in range(4):
                            h = 4 * hg + j
                            # start=True clears has_written for the whole
                            # bank: only the first slice may issue it, the
                            # second slice inherits the cleared bits.
                            nc.tensor.matmul(
                                av2[j // 2][:, j % 2, :],
                                v_st[kt][:, 33 * h:33 * h + 33],
                                pr[:, 256 * j:256 * (j + 1)],
                                start=(kt == 0 and j % 2 == 0),
                                stop=(kt == NKT - 1))

                    prev = None
                    for kt in range(NKT):
                        # ---- woven projection filler (PE stays dense) ----
                        if hg == 0 and qc == 0:
                            if kt == 0:
                                for g2 in range(2):
                                    for c2 in range(2):
                                        qT_unit(g2, c2)
                                kT_unit(0, 0)
                            elif kt % 4 == 0:
                                kT_unit(0, kt // 4)
                            v_unit(kt)
                        elif hg == 0 and qc == 1 and kt % 4 == 0:
                            kT_unit(1, kt // 4)

                        S = scps.tile([128, 4, 256], F32, tag="S",
                                      name=f"S_{hg}_{qc}_{kt}")
                        for j in range(4):
                            nc.tensor.matmul(
                                S[:, j, :],
                                kT_c[hg][kt // 4][:, 128 * (kt % 4):128 * (kt % 4 + 1)],
                                qT_pad[4 * hg + j][:, QC * qc:QC * (qc + 1)],
                                start=True, stop=True)
                        pr = prpool.tile([128, 4 * QC], F16, tag="pr",
                                         name=f"pr_{hg}_{qc}_{kt}")
                        nc.scalar.activation(out=pr, in_=S[:, :, :],
                                             func=Exp, scale=SCALE)
                        if prev is not None:
                            emit_av(prev, kt - 1)
                        prev = pr
                    emit_av(prev, NKT - 1)

                    for p in range(2):
                        a = avsb_pool.tile([33, 512], F32, tag="avsb",
                                           name=f"avsb_{hg}_{qc}_{p}")
                        nc.vector.tensor_copy(a, av2[p][:, :, :])
                        nc.sync.dma_start(out=den_dram[hg, qc, p, :],
                                          in_=a[32:33, :])
                        av_sb_all[(qc, p)] = a

                # batched reciprocal for this head-group: 4qc x 2p x 512 = [128,32]
                denb = norm_pool.tile([128, 32], F32, tag="denb", name=f"denb{hg}")
                nc.sync.dma_start(
                    out=denb,
                    in_=den_dram[hg, :, :, :].rearrange("a b c -> (a b c)")
                    .rearrange("(p f) -> p f", p=128))
                recb = norm_pool.tile([128, 32], F32, tag="recb", name=f"recb{hg}")
                nc.vector.reciprocal(recb, denb)
                nc.sync.dma_start(
                    out=recip_dram[hg, :, :, :].rearrange("a b c -> (a b c)")
                    .rearrange("(p f) -> p f", p=128),
                    in_=recb)

                for qc in range(NQC):
                    for j in range(4):
                        row = recip_dram[hg, qc, j // 2, 256 * (j % 2):256 * (j % 2) + 256]
                        bc = norm_pool.tile([32, 256], F32, tag="bc",
                                            name=f"bc_{hg}_{qc}_{j}")
                        nc.gpsimd.dma_start(
                            out=bc,
                            in_=bass.AP(tensor=row.tensor, offset=row.offset,
                                        ap=[[0, 32], row.ap[-1]]))
                        nc.vector.tensor_mul(
                            outT_c[hg][qc][32 * j:32 * (j + 1), :],
                            av_sb_all[(qc, j // 2)][0:32,
                                                   256 * (j % 2):256 * (j % 2) + 256],
                            bc)
                    # output projection as soon as both head-groups' outT
                    # columns for this q-chunk are normalized
                    if hg == 1:
                        for qt in (2 * qc, 2 * qc + 1):
                            po = kqps.tile([128, D], F32, tag="kq", name=f"po{qt}")
                            for g in range(2):
                                nc.tensor.matmul(
                                    po[:, :],
                                    outT_c[g][qt // 2][:, 128 * (qt % 2):128 * (qt % 2 + 1)],
                                    wo_sb[g][:, :],
                                    start=(g == 0), stop=(g == 1))
                            o = out_pool.tile([128, D], F32, tag="o", name=f"o{qt}")
                            nc.vector.tensor_copy(o, po[:, :])
                            nc.sync.dma_start(
                                out=out_d[128 * qt:128 * (qt + 1), :], in_=o)

    nc.compile()
    _BUILD_CACHE["nc"] = nc
    return nc


def _run(x, w_qkv, b_qkv, w_out, trace=False):
    nc = build()
    in_maps = []
    for c in range(8):
        bi, qh = c // 2, c % 2
        in_maps.append({
            "xT": np.ascontiguousarray(x[bi].T),
            "xqT": np.ascontiguousarray(x[bi, NQ * qh:NQ * (qh + 1)].T),
            "w_qkv": np.ascontiguousarray(w_qkv),
            "b_qkv": np.ascontiguousarray(b_qkv.reshape(1, 3 * D)),
            "w_out": np.ascontiguousarray(w_out),
        })
    res = run_bass_kernel_spmd(nc, in_maps, core_ids=list(range(8)), trace=trace)
    out = np.empty((B, N, D), dtype=np.float32)
    for c in range(8):
        bi, qh = c // 2, c % 2
        out[bi, NQ * qh:NQ * (qh + 1)] = res.results[c]["out"]
    return out, res


def kernel(x, w_qkv, b_qkv, w_out, b_out):
    x = np.asarray(x, dtype=np.float32)
    out, _ = _run(x, np.asarray(w_qkv, np.float32), np.asarray(b_qkv, np.float32),
                  np.asarray(w_out, np.float32))
    return out + np.asarray(b_out, np.float32)[None, None, :]


# revision 9
# speedup vs baseline: 2.0312x; 1.2782x over previous
"""Multi-head attention block (b=4, n=2048, d=256, h=8) on 8 TRN2 NeuronCores.

Sharding: core c handles (batch bi=c//2, query-half qh=c%2): it computes
K/V for the full sequence of its batch and Q for its 1024-row query half,
producing 1024 complete rows of the final output (host concatenates and
adds b_out; no cross-core reduction).

V2 design (all matmul operands fp16; PSUM fp32):
  - kT_all[hg] [128,2048]: 4 heads' K^T stacked (compact, whole-psum copies).
  - qT_pad[h] [128,1024]: per-head Q^T zero-padded to 128 partitions, so the
    scores matmul runs K=128 (lhsT = kT_all chunk; the zero rows of qT_pad
    mask the other heads). Single tile-position, full-array matmuls only.
  - q-chunks of 256: one scores psum tensor [128,4,256] (2 banks) holds all
    4 heads of a head-group for one k-tile; ONE exp [128,1024] per k-tile.
  - AV: [v|ones] lhsT [128,33] folds the softmax denominator (row 32);
    av accumulators [33,2,256] share a bank per head-pair -> psum fits in
    4 (scores) + 2 (av) + 2 (projection) = 8 banks.
  - QKV/V projection units are woven into the attention emission stream as
    PE filler to keep the tensor engine dense (HAM clock at 2+ GHz).
  - Normalization: denominator rows -> DRAM bounce -> batched reciprocal
    [128,32] -> broadcast-read -> DVE multiply -> outT (fp16).
"""
import numpy as np

import concourse.bacc as bacc
import concourse.bass as bass
import concourse.mybir as mybir
import concourse.tile as tile
from concourse.bass_utils import run_bass_kernel_spmd

F32 = mybir.dt.float32
F16 = mybir.dt.float16
Exp = mybir.ActivationFunctionType.Exp
Copy = mybir.ActivationFunctionType.Copy

B, N, D = 4, 2048, 256
H, DH = 8, 32
NQ = N // 2            # per-core query rows
SCALE = D ** -0.5      # 0.0625
NKT = N // 128         # 16 k-tiles
QC = 256               # q-chunk
NQC = NQ // QC         # 4 q-chunks per core

_BUILD_CACHE = {}


def build():
    if "nc" in _BUILD_CACHE:
        return _BUILD_CACHE["nc"]
    nc = bacc.Bacc()

    xT_d = nc.dram_tensor("xT", [D, N], F32, kind="ExternalInput")
    xqT_d = nc.dram_tensor("xqT", [D, NQ], F32, kind="ExternalInput")
    w_d = nc.dram_tensor("w_qkv", [D, 3 * D], F32, kind="ExternalInput")
    b_d = nc.dram_tensor("b_qkv", [1, 3 * D], F32, kind="ExternalInput")
    wo_d = nc.dram_tensor("w_out", [D, D], F32, kind="ExternalInput")
    out_d = nc.dram_tensor("out", [NQ, D], F32, kind="ExternalOutput")
    den_dram = nc.dram_tensor("den_scratch", [2, NQC, 2, 512], F32)
    recip_dram = nc.dram_tensor("recip_scratch", [2, NQC, 2, 512], F32)

    with tile.TileContext(nc) as tc:
        with (
            tc.tile_pool(name="persist", bufs=1) as persist,
            tc.tile_pool(name="probs", bufs=4) as prpool,
            tc.tile_pool(name="avsb", bufs=8) as avsb_pool,
            tc.tile_pool(name="norm", bufs=4) as norm_pool,
            tc.tile_pool(name="outsb", bufs=3) as out_pool,
            tc.tile_pool(name="kqps", bufs=2, space="PSUM") as kqps,
            tc.tile_pool(name="scps", bufs=2, space="PSUM") as scps,
            tc.tile_pool(name="avps", bufs=1, space="PSUM") as avps,
        ):
            # ---- persistent tiles / loads ----
            ones = persist.tile([1, 512], F16, name="ones")
            nc.vector.memset(ones, 1.0)

            w_sb = [persist.tile([128, 3 * D], F16, name=f"w{d2}") for d2 in range(2)]
            b_sb = persist.tile([1, 3 * D], F16, name="b_sb")
            xT_sb = [persist.tile([128, N], F16, name=f"xT{d2}") for d2 in range(2)]
            xqT_sb = [persist.tile([128, NQ], F16, name=f"xq{d2}") for d2 in range(2)]
            wo_sb = [persist.tile([128, D], F16, name=f"wo{g}") for g in range(2)]
            for d2 in range(2):
                nc.gpsimd.dma_start(out=w_sb[d2], in_=w_d[128 * d2:128 * (d2 + 1), :])
                nc.gpsimd.dma_start(out=xqT_sb[d2], in_=xqT_d[128 * d2:128 * (d2 + 1), :])
            nc.gpsimd.dma_start(out=b_sb, in_=b_d[:, :])
            for d2 in range(2):
                nc.gpsimd.dma_start(out=xT_sb[d2], in_=xT_d[128 * d2:128 * (d2 + 1), :])
            for g in range(2):
                nc.gpsimd.dma_start(out=wo_sb[g], in_=wo_d[128 * g:128 * (g + 1), :])

            # per-chunk tiles: a chunk is fully written before first read, so
            # tile-granular RAW tracking cannot create emission-order cycles
            kT_c = [[persist.tile([128, 512], F16, name=f"kT{g}_{c}")
                     for c in range(4)] for g in range(2)]
            qT_pad = [persist.tile([128, NQ], F16, name=f"qT{h}") for h in range(H)]
            v_st = [persist.tile([128, H * 33], F16, name=f"vst{s}")
                    for s in range(NKT)]
            outT_c = [[persist.tile([128, 256], F16, name=f"outT{g}_{c}")
                       for c in range(NQC)] for g in range(2)]
            for h in range(H):
                nc.gpsimd.memset(qT_pad[h], 0.0)
            for s in range(NKT):
                nc.gpsimd.memset(v_st[s], 1.0)

            # ---- projection units (emitted woven into attention) ----
            def qT_unit(hg, c):
                """q^T for head-group hg, seq chunk c (512 wide)."""
                p = kqps.tile([128, 512], F32, tag="kq", name=f"kqq_{hg}_{c}")
                for d2 in range(2):
                    nc.tensor.matmul(
                        p[:, :], w_sb[d2][:, 128 * hg:128 * (hg + 1)],
                        xqT_sb[d2][:, 512 * c:512 * (c + 1)],
                        start=(d2 == 0), stop=False)
                nc.tensor.matmul(
                    p[:, :], b_sb[:, 128 * hg:128 * (hg + 1)], ones[:, :],
                    start=False, stop=True)
                for j in range(4):
                    dst = qT_pad[4 * hg + j][32 * j:32 * (j + 1),
                                             512 * c:512 * (c + 1)]
                    if j % 2 == 0:
                        nc.vector.tensor_copy(out=dst, in_=p[32 * j:32 * (j + 1), :])
                    else:
                        nc.scalar.activation(out=dst, in_=p[32 * j:32 * (j + 1), :],
                                             func=Copy)

            def kT_unit(hg, c):
                """k^T for head-group hg, seq chunk c (512 wide)."""
                p = kqps.tile([128, 512], F32, tag="kq", name=f"kqk_{hg}_{c}")
                for d2 in range(2):
                    nc.tensor.matmul(
                        p[:, :], w_sb[d2][:, D + 128 * hg:D + 128 * (hg + 1)],
                        xT_sb[d2][:, 512 * c:512 * (c + 1)],
                        start=(d2 == 0), stop=False)
                nc.tensor.matmul(
                    p[:, :], b_sb[:, D + 128 * hg:D + 128 * (hg + 1)], ones[:, :],
                    start=False, stop=True)
                nc.scalar.activation(out=kT_c[hg][c][:, :], in_=p[:, :], func=Copy)

            def v_unit(st):
                """v rows for seq tile st (128 wide), all 8 heads + ones col."""
                p = kqps.tile([128, D], F32, tag="kq", name=f"vv_{st}")
                for d2 in range(2):
                    nc.tensor.matmul(
                        p[:, :], xT_sb[d2][:, 128 * st:128 * (st + 1)],
                        w_sb[d2][:, 2 * D:3 * D],
                        start=(d2 == 0), stop=False)
                nc.tensor.matmul(
                    p[:, :], ones[:, :128], b_sb[:, 2 * D:3 * D],
                    start=False, stop=True)
                nc.vector.tensor_copy(
                    out=v_st[st].rearrange("p (h c) -> p h c", h=H)[:, :, 0:32],
                    in_=p.rearrange("p (h c) -> p h c", h=H))

            # ---- attention ----
            for hg in range(2):
                av_sb_all = {}
                for qc in range(NQC):
                    av2 = [avps.tile([33, 2, 256], F32, tag=f"av{p}",
                                     name=f"av_{hg}_{qc}_{p}") for p in range(2)]

                    def emit_av(pr, kt):
                        for j in range(4):
                            h = 4 * hg + j
                            # start=True clears has_written for the whole
                            # bank: only the first slice may issue it, the
                            # second slice inherits the cleared bits.
                            nc.tensor.matmul(
                                av2[j // 2][:, j % 2, :],
                                v_st[kt][:, 33 * h:33 * h + 33],
                                pr[:, 256 * j:256 * (j + 1)],
                                start=(kt == 0 and j % 2 == 0),
                                stop=(kt == NKT - 1))

                    prev = None
                    for kt in range(NKT):
                        # ---- woven projection filler (PE stays dense) ----
                        if hg == 0 and qc == 0:
                            if kt == 0:
                                qT_unit(0, 0)
                                kT_unit(0, 0)
                            elif kt == 1:
                                qT_unit(0, 1)
                            elif kt % 4 == 0:
                                kT_unit(0, kt // 4)
                            v_unit(kt)
                        elif hg == 0 and qc == 1:
                            if kt in (0, 4):
                                qT_unit(1, kt // 4)
                            elif kt in (8, 12):
                                kT_unit(1, (kt - 8) // 4)
                        elif hg == 0 and qc == 2 and kt in (0, 4):
                            kT_unit(1, 2 + kt // 4)

                        S = scps.tile([128, 4, 256], F32, tag="S",
                                      name=f"S_{hg}_{qc}_{kt}")
                        for j in range(4):
                            nc.tensor.matmul(
                                S[:, j, :],
                                kT_c[hg][kt // 4][:, 128 * (kt % 4):128 * (kt % 4 + 1)],
                                qT_pad[4 * hg + j][:, QC * qc:QC * (qc + 1)],
                                start=True, stop=True)
                        pr = prpool.tile([128, 4 * QC], F16, tag="pr",
                                         name=f"pr_{hg}_{qc}_{kt}")
                        nc.scalar.activation(out=pr, in_=S[:, :, :],
                                             func=Exp, scale=SCALE)
                        if prev is not None:
                            emit_av(prev, kt - 1)
                        prev = pr
                    emit_av(prev, NKT - 1)

                    av_sb = []
                    for p in range(2):
                        a = avsb_pool.tile([33, 512], F32, tag="avsb",
                                           name=f"avsb_{hg}_{qc}_{p}")
                        nc.vector.tensor_copy(a, av2[p][:, :, :])
                        nc.sync.dma_start(out=den_dram[hg, qc, p, :],
                                          in_=a[32:33, :])
                        av_sb.append(a)

                    # per-qc normalize: batched reciprocal [128, 8], one
                    # 4-head broadcast read, 4 muls (+ outproj when hg==1)
                    denb = norm_pool.tile([128, 8], F32, tag="denb",
                                          name=f"denb{hg}_{qc}")
                    nc.sync.dma_start(
                        out=denb,
                        in_=den_dram[hg, qc, :, :].rearrange("a c -> (a c)")
                        .rearrange("(p f) -> p f", p=128))
                    recb = norm_pool.tile([128, 8], F32, tag="recb",
                                          name=f"recb{hg}_{qc}")
                    nc.vector.reciprocal(recb, denb)
                    nc.sync.dma_start(
                        out=recip_dram[hg, qc, :, :].rearrange("a c -> (a c)")
                        .rearrange("(p f) -> p f", p=128),
                        in_=recb)
                    for j in range(4):
                        row = recip_dram[hg, qc, j // 2,
                                         256 * (j % 2):256 * (j % 2) + 256]
                        bc = norm_pool.tile([32, 256], F32, tag="bc",
                                            name=f"bc_{hg}_{qc}_{j}")
                        nc.gpsimd.dma_start(
                            out=bc,
                            in_=bass.AP(tensor=row.tensor, offset=row.offset,
                                        ap=[[0, 32], row.ap[-1]]))
                        nc.vector.tensor_mul(
                            outT_c[hg][qc][32 * j:32 * (j + 1), :],
                            av_sb[j // 2][0:32,
                                          256 * (j % 2):256 * (j % 2) + 256],
                            bc)
                    if hg == 1:
                        for qt in (2 * qc, 2 * qc + 1):
                            po = kqps.tile([128, D], F32, tag="kq", name=f"po{qt}")
                            for g in range(2):
                                nc.tensor.matmul(
                                    po[:, :],
                                    outT_c[g][qt // 2][:, 128 * (qt % 2):128 * (qt % 2 + 1)],
                                    wo_sb[g][:, :],
                                    start=(g == 0), stop=(g == 1))
                            o = out_pool.tile([128, D], F32, tag="o", name=f"o{qt}")
                            nc.vector.tensor_copy(o, po[:, :])
                            nc.sync.dma_start(
                                out=out_d[128 * qt:128 * (qt + 1), :], in_=o)

    nc.compile()
    _BUILD_CACHE["nc"] = nc
    return nc


def _run(x, w_qkv, b_qkv, w_out, trace=False):
    nc = build()
    in_maps = []
    for c in range(8):
        bi, qh = c // 2, c % 2
        in_maps.append({
            "xT": np.ascontiguousarray(x[bi].T),
            "xqT": np.ascontiguousarray(x[bi, NQ * qh:NQ * (qh + 1)].T),
            "w_qkv": np.ascontiguousarray(w_qkv),
            "b_qkv": np.ascontiguousarray(b_qkv.reshape(1, 3 * D)),
            "w_out": np.ascontiguousarray(w_out),
        })
    res = run_bass_kernel_spmd(nc, in_maps, core_ids=list(range(8)), trace=trace)
    out = np.empty((B, N, D), dtype=np.float32)
    for c in range(8):
        bi, qh = c // 2, c % 2
        out[bi, NQ * qh:NQ * (qh + 1)] = res.results[c]["out"]
    return out, res


def kernel(x, w_qkv, b_qkv, w_out, b_out):
    x = np.asarray(x, dtype=np.float32)
    out, _ = _run(x, np.asarray(w_qkv, np.float32), np.asarray(b_qkv, np.float32),
                  np.asarray(w_out, np.float32))
    return out + np.asarray(b_out, np.float32)[None, None, :]


# revision 10
# speedup vs baseline: 2.0539x; 1.0112x over previous
"""Multi-head attention block (b=4, n=2048, d=256, h=8) on 8 TRN2 NeuronCores.

Sharding: core c handles (batch bi=c//2, query-half qh=c%2): it computes
K/V for the full sequence of its batch and Q for its 1024-row query half,
producing 1024 complete rows of the final output (host concatenates and
adds b_out; no cross-core reduction).

V2 design (all matmul operands fp16; PSUM fp32):
  - kT_all[hg] [128,2048]: 4 heads' K^T stacked (compact, whole-psum copies).
  - qT_pad[h] [128,1024]: per-head Q^T zero-padded to 128 partitions, so the
    scores matmul runs K=128 (lhsT = kT_all chunk; the zero rows of qT_pad
    mask the other heads). Single tile-position, full-array matmuls only.
  - q-chunks of 256: one scores psum tensor [128,4,256] (2 banks) holds all
    4 heads of a head-group for one k-tile; ONE exp [128,1024] per k-tile.
  - AV: [v|ones] lhsT [128,33] folds the softmax denominator (row 32);
    av accumulators [33,2,256] share a bank per head-pair -> psum fits in
    4 (scores) + 2 (av) + 2 (projection) = 8 banks.
  - QKV/V projection units are woven into the attention emission stream as
    PE filler to keep the tensor engine dense (HAM clock at 2+ GHz).
  - Normalization: denominator rows -> DRAM bounce -> batched reciprocal
    [128,32] -> broadcast-read -> DVE multiply -> outT (fp16).
"""
import numpy as np

import concourse.bacc as bacc
import concourse.bass as bass
import concourse.mybir as mybir
import concourse.tile as tile
from concourse.bass_utils import run_bass_kernel_spmd

F32 = mybir.dt.float32
F16 = mybir.dt.float16
Exp = mybir.ActivationFunctionType.Exp
Copy = mybir.ActivationFunctionType.Copy

B, N, D = 4, 2048, 256
H, DH = 8, 32
NQ = N // 2            # per-core query rows
SCALE = D ** -0.5      # 0.0625
NKT = N // 128         # 16 k-tiles
QC = 256               # q-chunk
NQC = NQ // QC         # 4 q-chunks per core

_BUILD_CACHE = {}


def build():
    if "nc" in _BUILD_CACHE:
        return _BUILD_CACHE["nc"]
    nc = bacc.Bacc()

    xT_d = nc.dram_tensor("xT", [D, N], F32, kind="ExternalInput")
    xqT_d = nc.dram_tensor("xqT", [D, NQ], F32, kind="ExternalInput")
    w_d = nc.dram_tensor("w_qkv", [D, 3 * D], F32, kind="ExternalInput")
    b_d = nc.dram_tensor("b_qkv", [1, 3 * D], F32, kind="ExternalInput")
    wo_d = nc.dram_tensor("w_out", [D, D], F32, kind="ExternalInput")
    out_d = nc.dram_tensor("out", [NQ, D], F32, kind="ExternalOutput")
    den_dram = nc.dram_tensor("den_scratch", [2, NQC, 2, 512], F32)
    recip_dram = nc.dram_tensor("recip_scratch", [2, NQC, 2, 512], F32)

    with tile.TileContext(nc) as tc:
        with (
            tc.tile_pool(name="persist", bufs=1) as persist,
            tc.tile_pool(name="probs", bufs=4) as prpool,
            tc.tile_pool(name="avsb", bufs=3) as avsb_pool,
            tc.tile_pool(name="norm", bufs=4) as norm_pool,
            tc.tile_pool(name="outsb", bufs=3) as out_pool,
            tc.tile_pool(name="kqps", bufs=2, space="PSUM") as kqps,
            tc.tile_pool(name="scps", bufs=2, space="PSUM") as scps,
            tc.tile_pool(name="avps", bufs=1, space="PSUM") as avps,
        ):
            # ---- persistent tiles / loads ----
            ones = persist.tile([1, 512], F16, name="ones")
            nc.vector.memset(ones, 1.0)

            w_sb = [persist.tile([128, 3 * D], F16, name=f"w{d2}") for d2 in range(2)]
            b_sb = persist.tile([1, 3 * D], F16, name="b_sb")
            xT_sb = [persist.tile([128, N], F16, name=f"xT{d2}") for d2 in range(2)]
            xqT_sb = [persist.tile([128, NQ], F16, name=f"xq{d2}") for d2 in range(2)]
            wo_sb = [persist.tile([128, D], F16, name=f"wo{g}") for g in range(2)]
            for d2 in range(2):
                nc.gpsimd.dma_start(out=w_sb[d2], in_=w_d[128 * d2:128 * (d2 + 1), :])
                nc.gpsimd.dma_start(out=xqT_sb[d2], in_=xqT_d[128 * d2:128 * (d2 + 1), :])
            nc.gpsimd.dma_start(out=b_sb, in_=b_d[:, :])
            for d2 in range(2):
                nc.gpsimd.dma_start(out=xT_sb[d2], in_=xT_d[128 * d2:128 * (d2 + 1), :])
            for g in range(2):
                nc.gpsimd.dma_start(out=wo_sb[g], in_=wo_d[128 * g:128 * (g + 1), :])

            # per-chunk tiles: a chunk is fully written before first read, so
            # tile-granular RAW tracking cannot create emission-order cycles
            kT_c = [[persist.tile([128, 512], F16, name=f"kT{g}_{c}")
                     for c in range(4)] for g in range(2)]
            qT_pad = [persist.tile([128, NQ], F16, name=f"qT{h}") for h in range(H)]
            v_st = [persist.tile([128, H * 33], F16, name=f"vst{s}")
                    for s in range(NKT)]
            outT_c = [[persist.tile([128, 256], F16, name=f"outT{g}_{c}")
                       for c in range(NQC)] for g in range(2)]
            for h in range(H):
                nc.gpsimd.memset(qT_pad[h], 0.0)
            for s in range(NKT):
                nc.gpsimd.memset(v_st[s], 1.0)

            # ---- projection units (emitted woven into attention) ----
            def qT_unit(hg, c):
                """q^T for head-group hg, seq chunk c (512 wide)."""
                p = kqps.tile([128, 512], F32, tag="kq", name=f"kqq_{hg}_{c}")
                for d2 in range(2):
                    nc.tensor.matmul(
                        p[:, :], w_sb[d2][:, 128 * hg:128 * (hg + 1)],
                        xqT_sb[d2][:, 512 * c:512 * (c + 1)],
                        start=(d2 == 0), stop=False)
                nc.tensor.matmul(
                    p[:, :], b_sb[:, 128 * hg:128 * (hg + 1)], ones[:, :],
                    start=False, stop=True)
                for j in range(4):
                    dst = qT_pad[4 * hg + j][32 * j:32 * (j + 1),
                                             512 * c:512 * (c + 1)]
                    if j % 2 == 0:
                        nc.vector.tensor_copy(out=dst, in_=p[32 * j:32 * (j + 1), :])
                    else:
                        nc.scalar.activation(out=dst, in_=p[32 * j:32 * (j + 1), :],
                                             func=Copy)

            def kT_unit(hg, c):
                """k^T for head-group hg, seq chunk c (512 wide)."""
                p = kqps.tile([128, 512], F32, tag="kq", name=f"kqk_{hg}_{c}")
                for d2 in range(2):
                    nc.tensor.matmul(
                        p[:, :], w_sb[d2][:, D + 128 * hg:D + 128 * (hg + 1)],
                        xT_sb[d2][:, 512 * c:512 * (c + 1)],
                        start=(d2 == 0), stop=False)
                nc.tensor.matmul(
                    p[:, :], b_sb[:, D + 128 * hg:D + 128 * (hg + 1)], ones[:, :],
                    start=False, stop=True)
                nc.scalar.activation(out=kT_c[hg][c][:, :], in_=p[:, :], func=Copy)

            def v_unit(st):
                """v rows for seq tile st (128 wide), all 8 heads + ones col."""
                p = kqps.tile([128, D], F32, tag="kq", name=f"vv_{st}")
                for d2 in range(2):
                    nc.tensor.matmul(
                        p[:, :], xT_sb[d2][:, 128 * st:128 * (st + 1)],
                        w_sb[d2][:, 2 * D:3 * D],
                        start=(d2 == 0), stop=False)
                nc.tensor.matmul(
                    p[:, :], ones[:, :128], b_sb[:, 2 * D:3 * D],
                    start=False, stop=True)
                nc.vector.tensor_copy(
                    out=v_st[st].rearrange("p (h c) -> p h c", h=H)[:, :, 0:32],
                    in_=p.rearrange("p (h c) -> p h c", h=H))

            # ---- attention ----
            for hg in range(2):
                av_sb_all = {}
                for qc in range(NQC):
                    av4 = avps.tile([33, 4, 256], F32, tag="av",
                                    name=f"av_{hg}_{qc}")

                    def emit_av(pr, kt):
                        for j in range(4):
                            h = 4 * hg + j
                            # start=True clears has_written for the whole
                            # bank: only the first slice in each bank may
                            # issue it; its sibling inherits the clear.
                            nc.tensor.matmul(
                                av4[:, j, :],
                                v_st[kt][:, 33 * h:33 * h + 33],
                                pr[:, 256 * j:256 * (j + 1)],
                                start=(kt == 0 and j % 2 == 0),
                                stop=(kt == NKT - 1))

                    prev = None
                    for kt in range(NKT):
                        # ---- woven projection filler (PE stays dense) ----
                        if hg == 0 and qc == 0:
                            if kt == 0:
                                qT_unit(0, 0)
                                kT_unit(0, 0)
                            elif kt == 1:
                                qT_unit(0, 1)
                            elif kt % 4 == 0:
                                kT_unit(0, kt // 4)
                            v_unit(kt)
                        elif hg == 0 and qc == 1:
                            if kt in (0, 4):
                                qT_unit(1, kt // 4)
                            elif kt in (8, 12):
                                kT_unit(1, (kt - 8) // 4)
                        elif hg == 0 and qc == 2 and kt in (0, 4):
                            kT_unit(1, 2 + kt // 4)

                        S = scps.tile([128, 4, 256], F32, tag="S",
                                      name=f"S_{hg}_{qc}_{kt}")
                        for j in range(4):
                            nc.tensor.matmul(
                                S[:, j, :],
                                kT_c[hg][kt // 4][:, 128 * (kt % 4):128 * (kt % 4 + 1)],
                                qT_pad[4 * hg + j][:, QC * qc:QC * (qc + 1)],
                                start=True, stop=True)
                        pr = prpool.tile([128, 4 * QC], F16, tag="pr",
                                         name=f"pr_{hg}_{qc}_{kt}")
                        nc.scalar.activation(out=pr, in_=S[:, :, :],
                                             func=Exp, scale=SCALE)
                        if prev is not None:
                            emit_av(prev, kt - 1)
                        prev = pr
                    emit_av(prev, NKT - 1)

                    a = avsb_pool.tile([33, 4, 256], F32, tag="avsb",
                                       name=f"avsb_{hg}_{qc}")
                    nc.vector.tensor_copy(a, av4[:, :, :])
                    nc.sync.dma_start(out=den_dram[hg, qc, :, :],
                                      in_=a[32:33, :, :])

                    # per-qc normalize: batched reciprocal [128, 8], one
                    # 4-head broadcast read, 4 muls (+ outproj when hg==1)
                    denb = norm_pool.tile([128, 8], F32, tag="denb",
                                          name=f"denb{hg}_{qc}")
                    nc.sync.dma_start(
                        out=denb,
                        in_=den_dram[hg, qc, :, :].rearrange("a c -> (a c)")
                        .rearrange("(p f) -> p f", p=128))
                    recb = norm_pool.tile([128, 8], F32, tag="recb",
                                          name=f"recb{hg}_{qc}")
                    nc.vector.reciprocal(recb, denb)
                    nc.sync.dma_start(
                        out=recip_dram[hg, qc, :, :].rearrange("a c -> (a c)")
                        .rearrange("(p f) -> p f", p=128),
                        in_=recb)
                    for j in range(4):
                        row = recip_dram[hg, qc, j // 2,
                                         256 * (j % 2):256 * (j % 2) + 256]
                        bc = norm_pool.tile([32, 256], F32, tag="bc",
                                            name=f"bc_{hg}_{qc}_{j}")
                        nc.gpsimd.dma_start(
                            out=bc,
                            in_=bass.AP(tensor=row.tensor, offset=row.offset,
                                        ap=[[0, 32], row.ap[-1]]))
                        nc.vector.tensor_mul(
                            outT_c[hg][qc][32 * j:32 * (j + 1), :],
                            a[0:32, j, :],
                            bc)
                    if hg == 1:
                        for qt in (2 * qc, 2 * qc + 1):
                            po = kqps.tile([128, D], F32, tag="kq", name=f"po{qt}")
                            for g in range(2):
                                nc.tensor.matmul(
                                    po[:, :],
                                    outT_c[g][qt // 2][:, 128 * (qt % 2):128 * (qt % 2 + 1)],
                                    wo_sb[g][:, :],
                                    start=(g == 0), stop=(g == 1))
                            o = out_pool.tile([128, D], F32, tag="o", name=f"o{qt}")
                            nc.vector.tensor_copy(o, po[:, :])
                            nc.sync.dma_start(
                                out=out_d[128 * qt:128 * (qt + 1), :], in_=o)

    nc.compile()
    _BUILD_CACHE["nc"] = nc
    return nc


def _run(x, w_qkv, b_qkv, w_out, trace=False):
    nc = build()
    in_maps = []
    for c in range(8):
        bi, qh = c // 2, c % 2
        in_maps.append({
            "xT": np.ascontiguousarray(x[bi].T),
            "xqT": np.ascontiguousarray(x[bi, NQ * qh:NQ * (qh + 1)].T),
            "w_qkv": np.ascontiguousarray(w_qkv),
            "b_qkv": np.ascontiguousarray(b_qkv.reshape(1, 3 * D)),
            "w_out": np.ascontiguousarray(w_out),
        })
    res = run_bass_kernel_spmd(nc, in_maps, core_ids=list(range(8)), trace=trace)
    out = np.empty((B, N, D), dtype=np.float32)
    for c in range(8):
        bi, qh = c // 2, c % 2
        out[bi, NQ * qh:NQ * (qh + 1)] = res.results[c]["out"]
    return out, res


def kernel(x, w_qkv, b_qkv, w_out, b_out):
    x = np.asarray(x, dtype=np.float32)
    out, _ = _run(x, np.asarray(w_qkv, np.float32), np.asarray(b_qkv, np.float32),
                  np.asarray(w_out, np.float32))
    return out + np.asarray(b_out, np.float32)[None, None, :]
